# revision 4
# baseline (speedup 1.0000x reference)
"""Trainium2 Bass kernel for nn_BinLoss (MAS binarization loss).

Algorithm
---------
reference = -sum(log(attn) * hard_alignment) / sum(hard_alignment)

Key identity: the masked log-sum over the backtracked MAS path equals the
forward DP value log_p[out_len-1, in_len-1] (standard Viterbi property:
backtracking reproduces the argmax path, whose score IS the DP cell).
sum(hard) == sum(out_lens).  So no backtracking is needed on device:
we run the (max,+) DP and read one cell per batch element.

Device DP layout (per core, 4 batch elements)
---------------------------------------------
DP recurrence over rows t:  lp[t, j] = la[t, j] + max(lp[t-1, j], lp[t-1, j-1])

Columns S=400 are split 16-per-partition over 25 partitions; each batch
element owns a 32-partition quadrant (partitions 25..31 are scratch).
Each partition keeps a K-wide halo of its left neighbor's columns so the
j-1 shift stays inside the partition; the halo is refreshed every K steps
with one stream_shuffle (per-quadrant partition rotate).  Row step = two
DVE ops (tensor_max of shifted views + tensor_add of the log row).

Masking is data-driven (host writes into its private shard copy):
  * rows >= out_len          -> attn 1.0  => la 0     (value freezes)
  * row out_len-1, j!=in-1   -> attn 0.0  => la -inf  (kills all but answer)
  * column pad [400, 512)    -> attn 0.0  => la -inf  (isolates quadrants)
Row 0 masking is the lp init: -1e30 everywhere except col 0 of each batch.
After the last row, every surviving finite cell of a quadrant equals the
answer, so a free-dim reduce_max + host max over the quadrant extracts it.

ln(attn) is computed on the scalar engine (Ln LUT) on chunked tiles that
are DMA'd straight from HBM into the halo-overlapped DP layout.
"""

import math
import sys

import numpy as np

sys.path.insert(0, "/opt/trn_rl_repo")

B, T, S = 32, 1600, 400
N_CORES = 8
BPC = B // N_CORES  # batch elements per core (4)

SC = 16            # columns per partition
PS = S // SC       # used partitions per batch element (25)
GROUP = 32         # partition quadrant per batch element
K = 8              # halo width (steps between refreshes)
W = SC + K         # tile width per partition
SPAD = 512         # padded row stride (cols 400..512 are mask pad)
FRONT = 64         # front pad elements (>= K)
BACK = 4096        # tail pad elements (covers scratch-partition overread)
FLAT = FRONT + BPC * T * SPAD + BACK

R = 200            # DP rows per DMA/ln chunk
NEG = -1.0e30

_prog_cache = {}


def _build_program(tmax):
    import concourse.bacc as bacc
    import concourse.bass as bass
    import concourse.mybir as mybir
    from concourse.tile import TileContext

    f32 = mybir.dt.float32
    nc = bacc.Bacc("TRN2", target_bir_lowering=False, debug=False)
    attn_d = nc.dram_tensor("attn", [FLAT], f32, kind="ExternalInput")
    out_d = nc.dram_tensor("res", [128, 1], f32, kind="ExternalOutput")

    nchunks = math.ceil(tmax / R)
    shuffle_mask = [31] + list(range(31))  # dest p <- src p-1 within quadrant

    with TileContext(nc) as tc:
        with (
            tc.tile_pool(name="la", bufs=3) as lap,
            tc.tile_pool(name="state", bufs=1) as sp,
        ):
            lp = sp.tile([128, W], f32, tag="lp")
            tmp = sp.tile([128, W], f32, tag="tmp")
            res = sp.tile([128, 1], f32, tag="res")

            for c in range(nchunks):
                r0 = c * R
                nr = min(tmax, r0 + R) - r0
                la = lap.tile([128, R * W], f32, tag="la")
                # per-batch DMA: (partition-in-quadrant 32)(row nr)(col W)
                for b in range(BPC):
                    src = bass.AP(
                        attn_d,
                        FRONT + b * T * SPAD + r0 * SPAD - K,
                        [[SC, GROUP], [SPAD, nr], [1, W]],
                    )
                    nc.sync.dma_start(
                        out=la[GROUP * b : GROUP * (b + 1), 0 : nr * W], in_=src
                    )
                nc.scalar.activation(
                    la[:, 0 : nr * W], la[:, 0 : nr * W],
                    mybir.ActivationFunctionType.Ln,
                )
                if c == 0:
                    nc.vector.memset(lp[:, :], NEG)
                    for b in range(BPC):
                        p = GROUP * b
                        nc.vector.tensor_copy(
                            lp[p : p + 1, K : K + 1], la[p : p + 1, K : K + 1]
                        )
                start_r = 1 if c == 0 else 0
                for r in range(start_r, nr):
                    i = r0 + r - 1  # step index, 0-based
                    if i > 0 and i % K == 0:
                        nc.vector.stream_shuffle(
                            lp[:, 0:K], lp[:, W - K : W], mask=shuffle_mask
                        )
                    nc.vector.tensor_max(
                        tmp[:, 1:W], lp[:, 1:W], lp[:, 0 : W - 1]
                    )
                    nc.vector.tensor_add(
                        lp[:, 1:W], tmp[:, 1:W],
                        la[:, r * W + 1 : (r + 1) * W],
                    )

            nc.vector.reduce_max(
                res[:, 0:1], lp[:, K:W], axis=mybir.AxisListType.X
            )
            nc.sync.dma_start(out=out_d.ap(), in_=res[:, 0:1])

    nc.compile()
    return nc


def _prep_shards(attn, in_lens, out_lens):
    """Per-core padded+masked flat input buffers."""
    in_maps = []
    for core in range(N_CORES):
        sh = np.zeros((BPC, T, SPAD), np.float32)
        sh[:, :, :S] = attn[core * BPC : (core + 1) * BPC, 0]
        for b in range(BPC):
            ob = int(out_lens[core * BPC + b])
            ib = int(in_lens[core * BPC + b])
            keep = sh[b, ob - 1, ib - 1]
            sh[b, ob - 1, :S] = 0.0          # la -> -inf
            sh[b, ob - 1, ib - 1] = keep
            sh[b, ob:, :S] = 1.0             # la -> 0
        flat = np.empty(FLAT, np.float32)
        flat[:FRONT] = 1.0
        flat[FRONT : FRONT + sh.size] = sh.ravel()
        flat[FRONT + sh.size :] = 1.0
        in_maps.append({"attn": flat})
    return in_maps


def _run(attn, in_lens, out_lens, trace=False):
    from concourse import bass_utils

    tmax = int(np.max(out_lens))
    if tmax not in _prog_cache:
        _prog_cache[tmax] = _build_program(tmax)
    nc = _prog_cache[tmax]
    in_maps = _prep_shards(attn, in_lens, out_lens)
    return bass_utils.run_bass_kernel_spmd(
        nc, in_maps, core_ids=list(range(N_CORES)), trace=trace
    )


def kernel(soft_attention, in_lens, out_lens, _trace=False):
    attn = np.asarray(soft_attention, dtype=np.float32)
    inl = np.asarray(in_lens)
    outl = np.asarray(out_lens)
    assert attn.shape == (B, 1, T, S), attn.shape

    res = _run(attn, inl, outl, trace=_trace)

    total = 0.0
    for core in range(N_CORES):
        v = res.results[core]["res"][:, 0]
        for b in range(BPC):
            total += float(np.max(v[GROUP * b : GROUP * b + PS]))
    count = float(np.sum(outl))
    out = np.array(-total / count, dtype=np.float32)
    if _trace:
        return out, res
    return out


# revision 5
# speedup vs baseline: 1.8856x; 1.8856x over previous
"""Trainium2 Bass kernel for nn_BinLoss (MAS binarization loss).

Algorithm
---------
reference = -sum(log(attn) * hard_alignment) / sum(hard_alignment)

Key identity: the masked log-sum over the backtracked MAS path equals the
forward DP value log_p[out_len-1, in_len-1] (Viterbi property: backtracking
reproduces the argmax path, whose score IS the DP cell), and
sum(hard) == sum(out_lens).  So no backtracking is needed on device.

Device DP (per core, 4 batch elements, data parallel over 8 cores)
------------------------------------------------------------------
DP over rows t:  lp[t, j] = la[t, j] + max(lp[t-1, j], lp[t-1, j-1])

Columns S=400 split 16-per-partition over 25 partitions; each batch element
owns a 32-partition quadrant (partitions 25..31 scratch).  Each partition
keeps a K-wide halo of its left neighbour's columns so the j-1 shift stays
in-partition; the halo is refreshed every K steps with one stream_shuffle
(per-quadrant partition rotate).  The row update is ONE custom DVE
instruction (hand-built uop program):

    out[k] = in1[k] + max(in0[k], in0[k-1])

where the lag-1 read comes from the swap flop (blk0 BYPASS(A=CURR_SWAP_OUT,
B=PREV_DELAY_0) with swap_enable: BYPASS emits A = previous element while
the swap latches B = current element).  A seed uop latches MAX_NEG.

Masking is data-driven (host writes into its private shard copy):
  * rows >= out_len          -> attn 1.0  => la 0     (value freezes/creeps)
  * row out_len-1, j!=in-1   -> attn 0.0  => la -inf  (kills all but answer)
  * column pad [400, 512)    -> attn 0.0  => la -inf  (isolates quadrants)
Row 0 masking is the lp init: -1e30 everywhere except col 0 of each batch.
After the last row every surviving finite cell of a quadrant equals the
answer: free-dim reduce_max + host max over the quadrant extracts it.

ln(attn) runs on the scalar engine (Ln LUT; Ln(0) = -inf on this HW) over
chunks DMA'd straight from HBM into the halo-overlapped DP layout.
"""

import math
import sys

import numpy as np

sys.path.insert(0, "/opt/trn_rl_repo")

B, T, S = 32, 1600, 400
N_CORES = 8
BPC = B // N_CORES  # batch elements per core (4)

SC = 16            # columns per partition
PS = S // SC       # used partitions per batch element (25)
GROUP = 32         # partition quadrant per batch element
K = 8              # halo width (steps between refreshes)
W = SC + K         # tile width per partition
SPAD = 512         # padded row stride (cols 400..512 are mask pad)
FRONT = 64         # front pad elements (>= K)
BACK = 4096        # tail pad elements (covers scratch-partition overread)
FLAT = FRONT + BPC * T * SPAD + BACK

R = 200            # max DP rows per DMA/ln chunk
NEG = -1.0e30

_prog_cache = {}
_fused_op = None


# --------------------------------------------------------------------------
# custom DVE op: out[k] = in1[k] + max(in0[k], in0[k-1])
# --------------------------------------------------------------------------
def _build_uops():
    from concourse.dve_uop import (
        DISABLE,
        ENABLE,
        AluInp,
        AluOp,
        InpSel,
        OutPath,
        OutSel,
        Trigger,
        UopConfig,
        UopDpConfig,
    )

    def dp_default():
        return [UopDpConfig() for _ in range(8)]

    seed = UopConfig()
    seed.enable_input(InpSel.SRC_0, 1)
    seed.enable_input(InpSel.SRC_1, 2)
    seed.enable_input(InpSel.MAX_NEG, 3)
    seed.trigger = (Trigger.COUNT, Trigger.NONE, Trigger.NONE)
    seed.repeat_count = 1
    seed.next_uop = (1, 0, 0)
    seed.require_inp0 = DISABLE
    seed.require_inp1 = DISABLE
    seed.datapath_config = dp_default()
    b0 = seed.datapath_config[0]
    b0.enable_alu(AluOp.BYPASS, AluInp.PREV_DELAY_2, AluInp.PREV_DELAY_2)
    b0.swap_enable = ENABLE
    b0.pass_through_delay(0, 1, 2)
    for k in range(1, 8):
        seed.datapath_config[k].pass_through_alu()
        seed.datapath_config[k].pass_through_delay(0, 1, 2)

    st = UopConfig()
    st.enable_input(InpSel.SRC_0, 1)   # lp -> PREV_DELAY_0 at blk0
    st.enable_input(InpSel.SRC_1, 2)   # la -> PREV_DELAY_1 at blk0
    st.enable_input(InpSel.MAX_NEG, 3)
    st.trigger = (Trigger.SRC_TENSOR_DONE, Trigger.NONE, Trigger.NONE)
    st.next_uop = (0, 0, 0)
    st.require_inp0 = ENABLE
    st.require_inp1 = ENABLE
    st.enable_output(OutSel.ALU_OUT, OutPath.WR0_LO)
    st.datapath_config = dp_default()
    d = st.datapath_config
    d[0].enable_alu(AluOp.BYPASS, AluInp.CURR_SWAP_OUT, AluInp.PREV_DELAY_0)
    d[0].swap_enable = ENABLE
    d[0].pass_through_delay(0, 1)
    d[1].enable_alu(AluOp.MAX, AluInp.PREV_ALU_OUT, AluInp.PREV_DELAY_0)
    d[1].pass_through_delay(1)
    d[2].enable_alu(AluOp.ADD, AluInp.PREV_ALU_OUT, AluInp.PREV_DELAY_1)
    for k in range(3, 8):
        d[k].pass_through_alu()
    return [seed, st]


class _MasStepOp:
    name = "MAS_STEP_ANT"
    subdim = False

    def __init__(self):
        from concourse.dve_spec import Spec, Src0, Src1

        self.spec = Spec(
            body=Src0 + Src1,
            reference=lambda in0, in1: in1
            + np.maximum(
                in0,
                np.concatenate(
                    [np.full_like(in0[..., :1], -3.4e38), in0[..., :-1]], axis=-1
                ),
            ),
        )
        self._cache = {}

    def compile(self, ver):
        from concourse.dve_uop import DveOpSpec

        if ver not in self._cache:
            from concourse.dve_ops import get_dve_sub_opcode

            self._cache[ver] = DveOpSpec(
                name=self.name,
                opcode=get_dve_sub_opcode(self.name),
                uops=_build_uops(),
                rd1_en=True,
            )
        return self._cache[ver]


def _get_fused_op():
    global _fused_op
    if _fused_op is None:
        import concourse.dve_ops as dve_ops

        _fused_op = _MasStepOp()
        if all(o.name != _fused_op.name for o in dve_ops.OPS):
            dve_ops.OPS.append(_fused_op)
            dve_ops._SUB_OPCODE_FOR_NAME[_fused_op.name] = (
                max(dve_ops._SUB_OPCODE_FOR_NAME.values()) + 1
            )
            assert dve_ops._SUB_OPCODE_FOR_NAME[_fused_op.name] < 0x20
    return _fused_op


# --------------------------------------------------------------------------
# program
# --------------------------------------------------------------------------
def _chunk_plan(tmax):
    """Progressive chunk sizes: small first chunks so the DP starts early."""
    plan = []
    r0 = 0
    for nr in (16, 16, 32, 64, 128):
        if r0 >= tmax:
            return plan
        nr = min(nr, tmax - r0)
        plan.append((r0, nr))
        r0 += nr
    while r0 < tmax:
        nr = min(R, tmax - r0)
        plan.append((r0, nr))
        r0 += nr
    return plan


def _build_program(tmax):
    import concourse.bacc as bacc
    import concourse.bass as bass
    import concourse.mybir as mybir
    from concourse.tile import TileContext

    op = _get_fused_op()
    f32 = mybir.dt.float32
    nc = bacc.Bacc("TRN2", target_bir_lowering=False, debug=False)
    attn_d = nc.dram_tensor("attn", [FLAT], f32, kind="ExternalInput")
    out_d = nc.dram_tensor("res", [128, 1], f32, kind="ExternalOutput")

    shuffle_mask = [31] + list(range(31))  # dest p <- src p-1 within quadrant

    with TileContext(nc) as tc:
        with (
            tc.tile_pool(name="la", bufs=3) as lap,
            tc.tile_pool(name="state", bufs=1) as sp,
        ):
            lp = sp.tile([128, W], f32, tag="lp")
            res = sp.tile([128, 1], f32, tag="res")

            for ci, (r0, nr) in enumerate(_chunk_plan(tmax)):
                la = lap.tile([128, R * W], f32, tag="la")
                # per-batch DMA: (partition-in-quadrant 32)(row nr)(col W)
                for b in range(BPC):
                    src = bass.AP(
                        attn_d,
                        FRONT + b * T * SPAD + r0 * SPAD - K,
                        [[SC, GROUP], [SPAD, nr], [1, W]],
                    )
                    nc.sync.dma_start(
                        out=la[GROUP * b : GROUP * (b + 1), 0 : nr * W], in_=src
                    )
                nc.scalar.activation(
                    la[:, 0 : nr * W], la[:, 0 : nr * W],
                    mybir.ActivationFunctionType.Ln,
                )
                if ci == 0:
                    nc.vector.memset(lp[:, :], NEG)
                    for b in range(BPC):
                        p = GROUP * b
                        nc.vector.tensor_copy(
                            lp[p : p + 1, K : K + 1], la[p : p + 1, K : K + 1]
                        )
                start_r = 1 if ci == 0 else 0
                for r in range(start_r, nr):
                    i = r0 + r - 1  # step index, 0-based
                    if i > 0 and i % K == 0:
                        nc.vector.stream_shuffle(
                            lp[:, 0:K], lp[:, W - K : W], mask=shuffle_mask
                        )
                    nc.vector._custom_dve(
                        op,
                        out=lp[:, 0:W],
                        in0=lp[:, 0:W],
                        in1=la[:, r * W : (r + 1) * W],
                    )

            nc.vector.reduce_max(
                res[:, 0:1], lp[:, K:W], axis=mybir.AxisListType.X
            )
            nc.sync.dma_start(out=out_d.ap(), in_=res[:, 0:1])

    nc.compile()
    return nc


def _prep_shards(attn, in_lens, out_lens):
    """Per-core padded+masked flat input buffers."""
    in_maps = []
    for core in range(N_CORES):
        sh = np.zeros((BPC, T, SPAD), np.float32)
        sh[:, :, :S] = attn[core * BPC : (core + 1) * BPC, 0]
        for b in range(BPC):
            ob = int(out_lens[core * BPC + b])
            ib = int(in_lens[core * BPC + b])
            keep = sh[b, ob - 1, ib - 1]
            sh[b, ob - 1, :S] = 0.0          # la -> -inf
            sh[b, ob - 1, ib - 1] = keep
            sh[b, ob:, :S] = 1.0             # la -> 0
        flat = np.empty(FLAT, np.float32)
        flat[:FRONT] = 1.0
        flat[FRONT : FRONT + sh.size] = sh.ravel()
        flat[FRONT + sh.size :] = 1.0
        in_maps.append({"attn": flat})
    return in_maps


def _run(attn, in_lens, out_lens, trace=False):
    from concourse import bass_utils

    tmax = int(np.max(out_lens))
    if tmax not in _prog_cache:
        _prog_cache[tmax] = _build_program(tmax)
    nc = _prog_cache[tmax]
    in_maps = _prep_shards(attn, in_lens, out_lens)
    return bass_utils.run_bass_kernel_spmd(
        nc, in_maps, core_ids=list(range(N_CORES)), trace=trace
    )


def kernel(soft_attention, in_lens, out_lens, _trace=False):
    attn = np.asarray(soft_attention, dtype=np.float32)
    inl = np.asarray(in_lens)
    outl = np.asarray(out_lens)
    assert attn.shape == (B, 1, T, S), attn.shape

    res = _run(attn, inl, outl, trace=_trace)

    total = 0.0
    for core in range(N_CORES):
        v = res.results[core]["res"][:, 0]
        for b in range(BPC):
            total += float(np.max(v[GROUP * b : GROUP * b + PS]))
    count = float(np.sum(outl))
    out = np.array(-total / count, dtype=np.float32)
    if _trace:
        return out, res
    return out


# revision 8
# speedup vs baseline: 2.1186x; 1.1236x over previous
"""Trainium2 Bass kernel for nn_BinLoss (MAS binarization loss).

Algorithm
---------
reference = -sum(log(attn) * hard_alignment) / sum(hard_alignment)

Key identity: the masked log-sum over the backtracked MAS path equals the
forward DP value log_p[out_len-1, in_len-1] (Viterbi property: backtracking
reproduces the argmax path, whose score IS the DP cell), and
sum(hard) == sum(out_lens).  So no backtracking is needed on device.

Device DP (per core, 4 batch elements, data parallel over 8 cores)
------------------------------------------------------------------
DP over rows t:  lp[t, j] = la[t, j] + max(lp[t-1, j], lp[t-1, j-1])

Columns S=400 split 16-per-partition over 25 partitions; each batch element
owns a 32-partition quadrant (partitions 25..31 scratch).  Each partition
keeps a K-wide halo of its left neighbour's columns so the j-1 shift stays
in-partition; the halo is refreshed every K steps with one stream_shuffle
(per-quadrant partition rotate).  The row update is ONE custom DVE
instruction (hand-built uop program):

    out[k] = in1[k] + max(in0[k], in0[k-1])

where the lag-1 read comes from the swap flop (blk0 BYPASS(A=CURR_SWAP_OUT,
B=PREV_DELAY_0) with swap_enable: BYPASS emits A = previous element while
the swap latches B = current element).  A seed uop latches MAX_NEG.

Masking is data-driven (host writes into its private shard copy):
  * rows >= out_len          -> attn 1.0  => la 0     (value freezes/creeps)
  * row out_len-1, j!=in-1   -> attn 0.0  => la -inf  (kills all but answer)
  * column pad [400, 512)    -> attn 0.0  => la -inf  (isolates quadrants)
Row 0 masking is the lp init: -1e30 everywhere except col 0 of each batch.
After the last row every surviving finite cell of a quadrant equals the
answer: free-dim reduce_max + host max over the quadrant extracts it.

ln(attn) runs on the scalar engine (Ln LUT; Ln(0) = -inf on this HW) over
chunks DMA'd straight from HBM into the halo-overlapped DP layout.
"""

import math
import sys

import numpy as np

sys.path.insert(0, "/opt/trn_rl_repo")

B, T, S = 32, 1600, 400
N_CORES = 8
BPC = B // N_CORES  # batch elements per core (4)

SC = 16            # columns per partition
PS = S // SC       # used partitions per batch element (25)
GROUP = 32         # partition quadrant per batch element
K = 8              # halo width (steps between refreshes)
W = SC + K         # tile width per partition
SPAD = 512         # padded row stride (cols 400..512 are mask pad)
FRONT = 64         # front pad elements (>= K)
BACK = 4096        # tail pad elements (covers scratch-partition overread)
FLAT = FRONT + BPC * T * SPAD + BACK

R = 200            # max DP rows per DMA/ln chunk
NEG = -1.0e30

_prog_cache = {}
_fused_op = None


# --------------------------------------------------------------------------
# custom DVE op: out[k] = in1[k] + max(in0[k], in0[k-1])
# --------------------------------------------------------------------------
def _build_uops():
    from concourse.dve_uop import (
        DISABLE,
        ENABLE,
        AluInp,
        AluOp,
        InpSel,
        OutPath,
        OutSel,
        Trigger,
        UopConfig,
        UopDpConfig,
    )

    def dp_default():
        return [UopDpConfig() for _ in range(8)]

    seed = UopConfig()
    seed.enable_input(InpSel.SRC_0, 1)
    seed.enable_input(InpSel.SRC_1, 2)
    seed.enable_input(InpSel.MAX_NEG, 3)
    seed.trigger = (Trigger.COUNT, Trigger.NONE, Trigger.NONE)
    seed.repeat_count = 1
    seed.next_uop = (1, 0, 0)
    seed.require_inp0 = DISABLE
    seed.require_inp1 = DISABLE
    seed.datapath_config = dp_default()
    b0 = seed.datapath_config[0]
    b0.enable_alu(AluOp.BYPASS, AluInp.PREV_DELAY_2, AluInp.PREV_DELAY_2)
    b0.swap_enable = ENABLE
    b0.pass_through_delay(0, 1, 2)
    for k in range(1, 8):
        seed.datapath_config[k].pass_through_alu()
        seed.datapath_config[k].pass_through_delay(0, 1, 2)

    st = UopConfig()
    st.enable_input(InpSel.SRC_0, 1)   # lp -> PREV_DELAY_0 at blk0
    st.enable_input(InpSel.SRC_1, 2)   # la -> PREV_DELAY_1 at blk0
    st.enable_input(InpSel.MAX_NEG, 3)
    st.trigger = (Trigger.SRC_TENSOR_DONE, Trigger.NONE, Trigger.NONE)
    st.next_uop = (0, 0, 0)
    st.require_inp0 = ENABLE
    st.require_inp1 = ENABLE
    st.enable_output(OutSel.ALU_OUT, OutPath.WR0_LO)
    st.datapath_config = dp_default()
    d = st.datapath_config
    d[0].enable_alu(AluOp.BYPASS, AluInp.CURR_SWAP_OUT, AluInp.PREV_DELAY_0)
    d[0].swap_enable = ENABLE
    d[0].pass_through_delay(0, 1)
    d[1].enable_alu(AluOp.MAX, AluInp.PREV_ALU_OUT, AluInp.PREV_DELAY_0)
    d[1].pass_through_delay(1)
    d[2].enable_alu(AluOp.ADD, AluInp.PREV_ALU_OUT, AluInp.PREV_DELAY_1)
    for k in range(3, 8):
        d[k].pass_through_alu()
    return [seed, st]


def _build_pair_uops():
    """2-row op: out2[k] = la2[k] + max(out1[k], out1[k-1]),
    out1[k] = la1[k] + max(lp[k], lp[k-1]).  Elements alternate uop A
    (computes out1) / uop B (computes out2, writes); per-stage config
    travels with each element.  in0 = lp duplicated [P,W,2]; in1 =
    (la1[k], la2[k]) interleaved [P,W,2]."""
    from concourse.dve_uop import (
        DISABLE,
        ENABLE,
        AluInp,
        AluOp,
        InpSel,
        OutPath,
        OutSel,
        Trigger,
        UopConfig,
        UopDpConfig,
    )

    def dp_default():
        return [UopDpConfig() for _ in range(8)]

    seed = UopConfig()
    seed.enable_input(InpSel.SRC_0, 1)
    seed.enable_input(InpSel.SRC_1, 2)
    seed.enable_input(InpSel.MAX_NEG, 3)
    seed.trigger = (Trigger.COUNT, Trigger.NONE, Trigger.NONE)
    seed.repeat_count = 4
    seed.next_uop = (1, 0, 0)
    seed.require_inp0 = DISABLE
    seed.require_inp1 = DISABLE
    seed.datapath_config = dp_default()
    sd = seed.datapath_config
    sd[0].enable_alu(AluOp.BYPASS, AluInp.PREV_DELAY_2, AluInp.PREV_DELAY_2)
    sd[0].swap_enable = ENABLE
    sd[0].pass_through_delay(0, 1, 2)
    for k in range(1, 8):
        sd[k].pass_through_alu()
        sd[k].pass_through_delay(0, 1, 2)
    sd[3].enable_alu(AluOp.BYPASS, AluInp.PREV_DELAY_2, AluInp.PREV_DELAY_2)
    sd[3].swap_enable = ENABLE

    def phase_uop(is_a):
        u = UopConfig()
        u.enable_input(InpSel.SRC_0, 1)
        u.enable_input(InpSel.SRC_1, 2)
        u.enable_input(InpSel.MAX_NEG, 3)
        u.trigger = (Trigger.SRC_TENSOR_DONE, Trigger.COUNT, Trigger.NONE)
        u.repeat_count = 1
        u.next_uop = (0, 2 if is_a else 1, 0)
        u.require_inp0 = ENABLE
        u.require_inp1 = ENABLE
        d = u.datapath_config = dp_default()
        d[0].enable_alu(AluOp.BYPASS, AluInp.CURR_SWAP_OUT, AluInp.PREV_DELAY_0)
        d[0].swap_enable = ENABLE if is_a else DISABLE
        d[0].pass_through_delay(0, 1)
        if is_a:
            d[1].enable_alu(AluOp.MAX, AluInp.PREV_ALU_OUT, AluInp.PREV_DELAY_0)
            d[1].pass_through_delay(1)
            d[2].enable_alu(AluOp.ADD, AluInp.PREV_ALU_OUT, AluInp.PREV_DELAY_1)
            d[3].enable_alu(AluOp.BYPASS, AluInp.CURR_SWAP_OUT, AluInp.PREV_ALU_OUT)
            d[3].swap_enable = ENABLE
            d[4].pass_through_alu()
            d[5].pass_through_alu()
        else:
            d[1].pass_through_alu()
            d[1].pass_through_delay(1)
            d[2].pass_through_alu()
            d[2].pass_through_delay(1)
            d[3].enable_alu(AluOp.BYPASS, AluInp.CURR_SWAP_OUT, AluInp.CURR_SWAP_OUT)
            d[3].pass_through_delay(1)
            d[4].enable_alu(AluOp.MAX, AluInp.PREV_ALU_OUT, AluInp.CURR_ALU_OUT)
            d[4].pass_through_delay(1)
            d[5].enable_alu(AluOp.ADD, AluInp.PREV_ALU_OUT, AluInp.PREV_DELAY_1)
        d[6].pass_through_alu()
        d[7].pass_through_alu()
        if not is_a:
            u.enable_output(OutSel.ALU_OUT, OutPath.WR0_LO)
        return u

    return [seed, phase_uop(True), phase_uop(False)]


class _CustomOp:
    subdim = False

    def __init__(self, name, build):
        from concourse.dve_spec import Spec, Src0, Src1

        self.name = name
        self._build = build
        self.spec = Spec(body=Src0 + Src1, reference=None)
        self._cache = {}

    def compile(self, ver):
        from concourse.dve_uop import DveOpSpec

        if ver not in self._cache:
            from concourse.dve_ops import get_dve_sub_opcode

            self._cache[ver] = DveOpSpec(
                name=self.name,
                opcode=get_dve_sub_opcode(self.name),
                uops=self._build(),
                rd1_en=True,
            )
        return self._cache[ver]


def _register_op(name, build):
    import concourse.dve_ops as dve_ops

    for o in dve_ops.OPS:
        if o.name == name:
            return o
    op = _CustomOp(name, build)
    dve_ops.OPS.append(op)
    dve_ops._SUB_OPCODE_FOR_NAME[name] = (
        max(dve_ops._SUB_OPCODE_FOR_NAME.values()) + 1
    )
    assert dve_ops._SUB_OPCODE_FOR_NAME[name] < 0x20
    return op


def _get_fused_op():
    return _register_op("MAS_STEP_ANT", _build_uops)


def _get_pair_op():
    return _register_op("MAS_PAIR_ANT", _build_pair_uops)


# --------------------------------------------------------------------------
# program
# --------------------------------------------------------------------------
def _chunk_plan(tmax):
    """Progressive chunk sizes so the DP starts early.  First chunk is 17
    rows (t=0..16 -> 16 DP steps); later chunks even-sized, so DP step
    parity stays aligned with row pairs and K=8 refresh boundaries."""
    plan = []
    r0 = 0
    for nr in (17, 16, 32, 64, 128):
        if r0 >= tmax:
            return plan
        nr = min(nr, tmax - r0)
        plan.append((r0, nr))
        r0 += nr
    while r0 < tmax:
        nr = min(R, tmax - r0)
        plan.append((r0, nr))
        r0 += nr
    return plan


def _build_program(tmax):
    import concourse.bacc as bacc
    import concourse.bass as bass
    import concourse.mybir as mybir
    from concourse.tile import TileContext

    op1 = _get_fused_op()
    op2 = _get_pair_op()
    f32 = mybir.dt.float32
    nc = bacc.Bacc("TRN2", target_bir_lowering=False, debug=False)
    attn_d = nc.dram_tensor("attn", [FLAT], f32, kind="ExternalInput")
    out_d = nc.dram_tensor("res", [128, 1], f32, kind="ExternalOutput")

    shuffle_mask = [31] + list(range(31))  # dest p <- src p-1 within quadrant

    with TileContext(nc) as tc:
        with (
            tc.tile_pool(name="la", bufs=3) as lap,
            tc.tile_pool(name="state", bufs=1) as sp,
        ):
            lp = sp.tile([128, W], f32, tag="lp")
            res = sp.tile([128, 1], f32, tag="res")
            in0_pair = lp[:, 0:W].unsqueeze(2).broadcast_to([128, W, 2])

            for ci, (r0, nr) in enumerate(_chunk_plan(tmax)):
                la = lap.tile([128, R * W], f32, tag="la")
                # per-batch DMAs: real cols on partitions 0..24, plus the
                # quadrant-isolating partition 31 reading the -inf column pad
                for b in range(BPC):
                    base = FRONT + b * T * SPAD + r0 * SPAD - K
                    nc.sync.dma_start(
                        out=la[GROUP * b : GROUP * b + PS, 0 : nr * W],
                        in_=bass.AP(
                            attn_d, base, [[SC, PS], [SPAD, nr], [1, W]]
                        ),
                    )
                    nc.sync.dma_start(
                        out=la[GROUP * b + 31 : GROUP * b + 32, 0 : nr * W],
                        in_=bass.AP(
                            attn_d, base + 31 * SC, [[SPAD, nr], [1, W]]
                        ),
                    )
                nc.scalar.activation(
                    la[:, 0 : nr * W], la[:, 0 : nr * W],
                    mybir.ActivationFunctionType.Ln,
                )
                if ci == 0:
                    nc.vector.memset(lp[:, :], NEG)
                    for b in range(BPC):
                        p = GROUP * b
                        nc.vector.tensor_copy(
                            lp[p : p + 1, K : K + 1], la[p : p + 1, K : K + 1]
                        )
                start_r = 1 if ci == 0 else 0
                r = start_r
                while r < nr:
                    i = r0 + r - 1  # step index, 0-based
                    if i > 0 and i % K == 0:
                        nc.vector.stream_shuffle(
                            lp[:, 0:K], lp[:, W - K : W], mask=shuffle_mask
                        )
                    if r + 1 < nr:
                        nc.vector._custom_dve(
                            op2,
                            out=lp[:, 0:W],
                            in0=in0_pair,
                            in1=la[:, r * W : (r + 2) * W].rearrange(
                                "p (two w) -> p w two", two=2
                            ),
                        )
                        r += 2
                    else:
                        nc.vector._custom_dve(
                            op1,
                            out=lp[:, 0:W],
                            in0=lp[:, 0:W],
                            in1=la[:, r * W : (r + 1) * W],
                        )
                        r += 1

            nc.vector.reduce_max(
                res[:, 0:1], lp[:, K:W], axis=mybir.AxisListType.X
            )
            nc.sync.dma_start(out=out_d.ap(), in_=res[:, 0:1])

    nc.compile()
    return nc


def _prep_shards(attn, in_lens, out_lens):
    """Per-core padded+masked flat input buffers."""
    in_maps = []
    for core in range(N_CORES):
        sh = np.zeros((BPC, T, SPAD), np.float32)
        sh[:, :, :S] = attn[core * BPC : (core + 1) * BPC, 0]
        for b in range(BPC):
            ob = int(out_lens[core * BPC + b])
            ib = int(in_lens[core * BPC + b])
            keep = sh[b, ob - 1, ib - 1]
            sh[b, ob - 1, :S] = 0.0          # la -> -inf
            sh[b, ob - 1, ib - 1] = keep
            sh[b, ob:, :S] = 1.0             # la -> 0
        flat = np.empty(FLAT, np.float32)
        flat[:FRONT] = 1.0
        flat[FRONT : FRONT + sh.size] = sh.ravel()
        flat[FRONT + sh.size :] = 1.0
        in_maps.append({"attn": flat})
    return in_maps


def _run(attn, in_lens, out_lens, trace=False):
    from concourse import bass_utils

    tmax = int(np.max(out_lens))
    if tmax not in _prog_cache:
        _prog_cache[tmax] = _build_program(tmax)
    nc = _prog_cache[tmax]
    in_maps = _prep_shards(attn, in_lens, out_lens)
    return bass_utils.run_bass_kernel_spmd(
        nc, in_maps, core_ids=list(range(N_CORES)), trace=trace
    )


def kernel(soft_attention, in_lens, out_lens, _trace=False):
    attn = np.asarray(soft_attention, dtype=np.float32)
    inl = np.asarray(in_lens)
    outl = np.asarray(out_lens)
    assert attn.shape == (B, 1, T, S), attn.shape

    res = _run(attn, inl, outl, trace=_trace)

    total = 0.0
    for core in range(N_CORES):
        v = res.results[core]["res"][:, 0]
        for b in range(BPC):
            total += float(np.max(v[GROUP * b : GROUP * b + PS]))
    count = float(np.sum(outl))
    out = np.array(-total / count, dtype=np.float32)
    if _trace:
        return out, res
    return out


# revision 11
# speedup vs baseline: 3.0020x; 1.4170x over previous
"""Trainium2 Bass kernel for nn_BinLoss (MAS binarization loss).

Algorithm
---------
reference = -sum(log(attn) * hard_alignment) / sum(hard_alignment)

Key identity: the masked log-sum over the backtracked MAS path equals the
forward DP value log_p[out_len-1, in_len-1] (Viterbi property: backtracking
reproduces the argmax path, whose score IS the DP cell), and
sum(hard) == sum(out_lens).  So no backtracking is needed on device.

Device DP (per core, 4 batch elements, data parallel over 8 cores)
------------------------------------------------------------------
DP over rows t:  lp[t, j] = la[t, j] + max(lp[t-1, j], lp[t-1, j-1])

Columns S=400 split 16-per-partition over 25 partitions; each batch element
owns a 32-partition quadrant (partitions 25..31 scratch).  Each partition
keeps a K-wide halo of its left neighbour's columns so the j-1 shift stays
in-partition; the halo is refreshed every K steps with one stream_shuffle
(per-quadrant partition rotate).  The row update is ONE custom DVE
instruction (hand-built uop program):

    out[k] = in1[k] + max(in0[k], in0[k-1])

where the lag-1 read comes from the swap flop (blk0 BYPASS(A=CURR_SWAP_OUT,
B=PREV_DELAY_0) with swap_enable: BYPASS emits A = previous element while
the swap latches B = current element).  A seed uop latches MAX_NEG.

Masking is data-driven (host writes into its private shard copy):
  * rows >= out_len          -> attn 1.0  => la 0     (value freezes/creeps)
  * row out_len-1, j!=in-1   -> attn 0.0  => la -inf  (kills all but answer)
  * column pad [400, 512)    -> attn 0.0  => la -inf  (isolates quadrants)
Row 0 masking is the lp init: -1e30 everywhere except col 0 of each batch.
After the last row every surviving finite cell of a quadrant equals the
answer: free-dim reduce_max + host max over the quadrant extracts it.

ln(attn) runs on the scalar engine (Ln LUT; Ln(0) = -inf on this HW) over
chunks DMA'd straight from HBM into the halo-overlapped DP layout.
"""

import math
import sys

import numpy as np

sys.path.insert(0, "/opt/trn_rl_repo")

B, T, S = 32, 1600, 400
N_CORES = 8
BPC = B // N_CORES  # batch elements per core (4)

SC = 16            # columns per partition
PS = S // SC       # used partitions per batch element (25)
GROUP = 32         # partition quadrant per batch element
K = 8              # halo width (steps between refreshes)
W = SC + K         # tile width per partition
FLAT = 128 * T * W  # host-pre-tiled input: [partition, t, w] contiguous

R = 200            # max DP rows per DMA/ln chunk
NEG = -1.0e30

_prog_cache = {}
_fused_op = None


# --------------------------------------------------------------------------
# custom DVE op: out[k] = in1[k] + max(in0[k], in0[k-1])
# --------------------------------------------------------------------------
def _build_uops():
    from concourse.dve_uop import (
        DISABLE,
        ENABLE,
        AluInp,
        AluOp,
        InpSel,
        OutPath,
        OutSel,
        Trigger,
        UopConfig,
        UopDpConfig,
    )

    def dp_default():
        return [UopDpConfig() for _ in range(8)]

    seed = UopConfig()
    seed.enable_input(InpSel.SRC_0, 1)
    seed.enable_input(InpSel.SRC_1, 2)
    seed.enable_input(InpSel.MAX_NEG, 3)
    seed.trigger = (Trigger.COUNT, Trigger.NONE, Trigger.NONE)
    seed.repeat_count = 1
    seed.next_uop = (1, 0, 0)
    seed.require_inp0 = DISABLE
    seed.require_inp1 = DISABLE
    seed.datapath_config = dp_default()
    b0 = seed.datapath_config[0]
    b0.enable_alu(AluOp.BYPASS, AluInp.PREV_DELAY_2, AluInp.PREV_DELAY_2)
    b0.swap_enable = ENABLE
    b0.pass_through_delay(0, 1, 2)
    for k in range(1, 8):
        seed.datapath_config[k].pass_through_alu()
        seed.datapath_config[k].pass_through_delay(0, 1, 2)

    st = UopConfig()
    st.enable_input(InpSel.SRC_0, 1)   # lp -> PREV_DELAY_0 at blk0
    st.enable_input(InpSel.SRC_1, 2)   # la -> PREV_DELAY_1 at blk0
    st.enable_input(InpSel.MAX_NEG, 3)
    st.trigger = (Trigger.SRC_TENSOR_DONE, Trigger.NONE, Trigger.NONE)
    st.next_uop = (0, 0, 0)
    st.require_inp0 = ENABLE
    st.require_inp1 = ENABLE
    st.enable_output(OutSel.ALU_OUT, OutPath.WR0_LO)
    st.datapath_config = dp_default()
    d = st.datapath_config
    d[0].enable_alu(AluOp.BYPASS, AluInp.CURR_SWAP_OUT, AluInp.PREV_DELAY_0)
    d[0].swap_enable = ENABLE
    d[0].pass_through_delay(0, 1)
    d[1].enable_alu(AluOp.MAX, AluInp.PREV_ALU_OUT, AluInp.PREV_DELAY_0)
    d[1].pass_through_delay(1)
    d[2].enable_alu(AluOp.ADD, AluInp.PREV_ALU_OUT, AluInp.PREV_DELAY_1)
    for k in range(3, 8):
        d[k].pass_through_alu()
    return [seed, st]


def _build_pair_uops():
    """2-row op: out2[k] = la2[k] + max(out1[k], out1[k-1]),
    out1[k] = la1[k] + max(lp[k], lp[k-1]).  Elements alternate uop A
    (computes out1) / uop B (computes out2, writes); per-stage config
    travels with each element.  in0 = lp duplicated [P,W,2]; in1 =
    (la1[k], la2[k]) interleaved [P,W,2]."""
    from concourse.dve_uop import (
        DISABLE,
        ENABLE,
        AluInp,
        AluOp,
        InpSel,
        OutPath,
        OutSel,
        Trigger,
        UopConfig,
        UopDpConfig,
    )

    def dp_default():
        return [UopDpConfig() for _ in range(8)]

    seed = UopConfig()
    seed.enable_input(InpSel.SRC_0, 1)
    seed.enable_input(InpSel.SRC_1, 2)
    seed.enable_input(InpSel.MAX_NEG, 3)
    seed.trigger = (Trigger.COUNT, Trigger.NONE, Trigger.NONE)
    seed.repeat_count = 4
    seed.next_uop = (1, 0, 0)
    seed.require_inp0 = DISABLE
    seed.require_inp1 = DISABLE
    seed.datapath_config = dp_default()
    sd = seed.datapath_config
    sd[0].enable_alu(AluOp.BYPASS, AluInp.PREV_DELAY_2, AluInp.PREV_DELAY_2)
    sd[0].swap_enable = ENABLE
    sd[0].pass_through_delay(0, 1, 2)
    for k in range(1, 8):
        sd[k].pass_through_alu()
        sd[k].pass_through_delay(0, 1, 2)
    sd[3].enable_alu(AluOp.BYPASS, AluInp.PREV_DELAY_2, AluInp.PREV_DELAY_2)
    sd[3].swap_enable = ENABLE

    def phase_uop(is_a):
        u = UopConfig()
        u.enable_input(InpSel.SRC_0, 1)
        u.enable_input(InpSel.SRC_1, 2)
        u.enable_input(InpSel.MAX_NEG, 3)
        u.trigger = (Trigger.SRC_TENSOR_DONE, Trigger.COUNT, Trigger.NONE)
        u.repeat_count = 1
        u.next_uop = (0, 2 if is_a else 1, 0)
        u.require_inp0 = ENABLE
        u.require_inp1 = ENABLE
        d = u.datapath_config = dp_default()
        d[0].enable_alu(AluOp.BYPASS, AluInp.CURR_SWAP_OUT, AluInp.PREV_DELAY_0)
        d[0].swap_enable = ENABLE if is_a else DISABLE
        d[0].pass_through_delay(0, 1)
        if is_a:
            d[1].enable_alu(AluOp.MAX, AluInp.PREV_ALU_OUT, AluInp.PREV_DELAY_0)
            d[1].pass_through_delay(1)
            d[2].enable_alu(AluOp.ADD, AluInp.PREV_ALU_OUT, AluInp.PREV_DELAY_1)
            d[3].enable_alu(AluOp.BYPASS, AluInp.CURR_SWAP_OUT, AluInp.PREV_ALU_OUT)
            d[3].swap_enable = ENABLE
            d[4].pass_through_alu()
            d[5].pass_through_alu()
        else:
            d[1].pass_through_alu()
            d[1].pass_through_delay(1)
            d[2].pass_through_alu()
            d[2].pass_through_delay(1)
            d[3].enable_alu(AluOp.BYPASS, AluInp.CURR_SWAP_OUT, AluInp.CURR_SWAP_OUT)
            d[3].pass_through_delay(1)
            d[4].enable_alu(AluOp.MAX, AluInp.PREV_ALU_OUT, AluInp.CURR_ALU_OUT)
            d[4].pass_through_delay(1)
            d[5].enable_alu(AluOp.ADD, AluInp.PREV_ALU_OUT, AluInp.PREV_DELAY_1)
        d[6].pass_through_alu()
        d[7].pass_through_alu()
        if not is_a:
            u.enable_output(OutSel.ALU_OUT, OutPath.WR0_LO)
        return u

    return [seed, phase_uop(True), phase_uop(False)]


class _CustomOp:
    subdim = False

    def __init__(self, name, build):
        from concourse.dve_spec import Spec, Src0, Src1

        self.name = name
        self._build = build
        self.spec = Spec(body=Src0 + Src1, reference=None)
        self._cache = {}

    def compile(self, ver):
        from concourse.dve_uop import DveOpSpec

        if ver not in self._cache:
            from concourse.dve_ops import get_dve_sub_opcode

            self._cache[ver] = DveOpSpec(
                name=self.name,
                opcode=get_dve_sub_opcode(self.name),
                uops=self._build(),
                rd1_en=True,
            )
        return self._cache[ver]


def _register_op(name, build):
    import concourse.dve_ops as dve_ops

    for o in dve_ops.OPS:
        if o.name == name:
            return o
    op = _CustomOp(name, build)
    dve_ops.OPS.append(op)
    dve_ops._SUB_OPCODE_FOR_NAME[name] = (
        max(dve_ops._SUB_OPCODE_FOR_NAME.values()) + 1
    )
    assert dve_ops._SUB_OPCODE_FOR_NAME[name] < 0x20
    return op


def _get_fused_op():
    return _register_op("MAS_STEP_ANT", _build_uops)


def _get_pair_op():
    return _register_op("MAS_PAIR_ANT", _build_pair_uops)


# --------------------------------------------------------------------------
# program
# --------------------------------------------------------------------------
def _chunk_plan(tmax):
    """Progressive chunk sizes so the DP starts early.  First chunk is 17
    rows (t=0..16 -> 16 DP steps); later chunks even-sized, so DP step
    parity stays aligned with row pairs and K=8 refresh boundaries."""
    plan = []
    r0 = 0
    for nr in (17, 16, 32, 64, 128):
        if r0 >= tmax:
            return plan
        nr = min(nr, tmax - r0)
        plan.append((r0, nr))
        r0 += nr
    while r0 < tmax:
        nr = min(R, tmax - r0)
        plan.append((r0, nr))
        r0 += nr
    return plan


def _build_program(tmax):
    import concourse.bacc as bacc
    import concourse.bass as bass
    import concourse.mybir as mybir
    from concourse.tile import TileContext

    op1 = _get_fused_op()
    op2 = _get_pair_op()
    f32 = mybir.dt.float32
    nc = bacc.Bacc("TRN2", target_bir_lowering=False, debug=False)
    attn_d = nc.dram_tensor("attn", [FLAT], f32, kind="ExternalInput")
    out_d = nc.dram_tensor("res", [128, 1], f32, kind="ExternalOutput")

    shuffle_mask = [31] + list(range(31))  # dest p <- src p-1 within quadrant

    with TileContext(nc) as tc:
        with (
            tc.tile_pool(name="la", bufs=3) as lap,
            tc.tile_pool(name="state", bufs=1) as sp,
        ):
            lp = sp.tile([128, W], f32, tag="lp")
            res = sp.tile([128, 1], f32, tag="res")
            in0_pair = lp[:, 0:W].unsqueeze(2).broadcast_to([128, W, 2])

            for ci, (r0, nr) in enumerate(_chunk_plan(tmax)):
                la = lap.tile([128, R * W], f32, tag="la")
                # host pre-tiled layout: one fully-contiguous run per partition
                nc.sync.dma_start(
                    out=la[:, 0 : nr * W],
                    in_=bass.AP(attn_d, r0 * W, [[T * W, 128], [1, nr * W]]),
                )
                nc.scalar.activation(
                    la[:, 0 : nr * W], la[:, 0 : nr * W],
                    mybir.ActivationFunctionType.Ln,
                )
                if ci == 0:
                    nc.vector.memset(lp[:, :], NEG)
                    for b in range(BPC):
                        p = GROUP * b
                        nc.vector.tensor_copy(
                            lp[p : p + 1, K : K + 1], la[p : p + 1, K : K + 1]
                        )
                start_r = 1 if ci == 0 else 0
                r = start_r
                while r < nr:
                    i = r0 + r - 1  # step index, 0-based
                    if i > 0 and i % K == 0:
                        nc.vector.stream_shuffle(
                            lp[:, 0:K], lp[:, W - K : W], mask=shuffle_mask
                        )
                    if r + 1 < nr:
                        nc.vector._custom_dve(
                            op2,
                            out=lp[:, 0:W],
                            in0=in0_pair,
                            in1=la[:, r * W : (r + 2) * W].rearrange(
                                "p (two w) -> p w two", two=2
                            ),
                        )
                        r += 2
                    else:
                        nc.vector._custom_dve(
                            op1,
                            out=lp[:, 0:W],
                            in0=lp[:, 0:W],
                            in1=la[:, r * W : (r + 1) * W],
                        )
                        r += 1

            nc.vector.reduce_max(
                res[:, 0:1], lp[:, K:W], axis=mybir.AxisListType.X
            )
            nc.sync.dma_start(out=out_d.ap(), in_=res[:, 0:1])

    nc.compile()
    return nc


def _prep_shards(attn, in_lens, out_lens):
    """Per-core masked + pre-tiled flat input buffers.

    Device layout [128, T, W]: partition 32b+s holds attn[b, t, s*16-K+w]
    (0.0 outside [0, 400) -> ln = -inf).  Partitions 25..31 of each quadrant
    stay 0.0, keeping quadrants isolated through the halo-rotate refresh."""
    in_maps = []
    pad = K + S + W  # padded column axis: [-K, S + W)
    for core in range(N_CORES):
        sh = np.zeros((BPC, T, pad), np.float32)
        sh[:, :, K : K + S] = attn[core * BPC : (core + 1) * BPC, 0]
        for b in range(BPC):
            ob = int(out_lens[core * BPC + b])
            ib = int(in_lens[core * BPC + b])
            keep = sh[b, ob - 1, K + ib - 1]
            sh[b, ob - 1, K : K + S] = 0.0   # la -> -inf
            sh[b, ob - 1, K + ib - 1] = keep
            sh[b, ob:, K : K + S] = 1.0      # la -> 0
        flat = np.zeros((128, T, W), np.float32)
        for b in range(BPC):
            win = np.lib.stride_tricks.sliding_window_view(sh[b], W, axis=1)
            flat[GROUP * b : GROUP * b + PS] = win[:, ::SC, :][:, :PS].transpose(
                1, 0, 2
            )
        in_maps.append({"attn": flat.ravel()})
    return in_maps


def _run(attn, in_lens, out_lens, trace=False):
    from concourse import bass_utils

    tmax = int(np.max(out_lens))
    if tmax not in _prog_cache:
        _prog_cache[tmax] = _build_program(tmax)
    nc = _prog_cache[tmax]
    in_maps = _prep_shards(attn, in_lens, out_lens)
    return bass_utils.run_bass_kernel_spmd(
        nc, in_maps, core_ids=list(range(N_CORES)), trace=trace
    )


def kernel(soft_attention, in_lens, out_lens, _trace=False):
    attn = np.asarray(soft_attention, dtype=np.float32)
    inl = np.asarray(in_lens)
    outl = np.asarray(out_lens)
    assert attn.shape == (B, 1, T, S), attn.shape

    res = _run(attn, inl, outl, trace=_trace)

    total = 0.0
    for core in range(N_CORES):
        v = res.results[core]["res"][:, 0]
        for b in range(BPC):
            total += float(np.max(v[GROUP * b : GROUP * b + PS]))
    count = float(np.sum(outl))
    out = np.array(-total / count, dtype=np.float32)
    if _trace:
        return out, res
    return out


# revision 16
# speedup vs baseline: 3.7949x; 1.2641x over previous
"""Trainium2 Bass kernel for nn_BinLoss (MAS binarization loss).

Algorithm
---------
reference = -sum(log(attn) * hard_alignment) / sum(hard_alignment)

Key identity: the masked log-sum over the backtracked MAS path equals the
forward DP value log_p[out_len-1, in_len-1] (Viterbi property: backtracking
reproduces the argmax path, whose score IS the DP cell), and
sum(hard) == sum(out_lens).  So no backtracking is needed on device.

Device DP (per core, 4 batch elements, data parallel over 8 cores)
------------------------------------------------------------------
DP over rows t:  lp[t, j] = la[t, j] + max(lp[t-1, j], lp[t-1, j-1])

Columns S=400 split 16-per-partition over 25 partitions; each batch element
owns a 32-partition quadrant (partitions 25..31 scratch).  Each partition
keeps a K-wide halo of its left neighbour's columns so the j-1 shift stays
in-partition; the halo is refreshed every K steps with one stream_shuffle
(per-quadrant partition rotate).  The row update is ONE custom DVE
instruction (hand-built uop program):

    out[k] = in1[k] + max(in0[k], in0[k-1])

where the lag-1 read comes from the swap flop (blk0 BYPASS(A=CURR_SWAP_OUT,
B=PREV_DELAY_0) with swap_enable: BYPASS emits A = previous element while
the swap latches B = current element).  A seed uop latches MAX_NEG.

Masking is data-driven (host writes into its private shard copy):
  * rows >= out_len          -> attn 1.0  => la 0     (value freezes/creeps)
  * row out_len-1, j!=in-1   -> attn 0.0  => la -inf  (kills all but answer)
  * column pad [400, 512)    -> attn 0.0  => la -inf  (isolates quadrants)
Row 0 masking is the lp init: -1e30 everywhere except col 0 of each batch.
After the last row every surviving finite cell of a quadrant equals the
answer: free-dim reduce_max + host max over the quadrant extracts it.

ln(attn) runs on the scalar engine (Ln LUT; Ln(0) = -inf on this HW) over
chunks DMA'd straight from HBM into the halo-overlapped DP layout.
"""

import math
import sys

import numpy as np

sys.path.insert(0, "/opt/trn_rl_repo")

B, T, S = 32, 1600, 400
N_CORES = 8
BPC = B // N_CORES  # batch elements per core (4)

SC = 16            # columns per partition
PS = S // SC       # used partitions per batch element (25)
GROUP = 32         # partition quadrant per batch element
K = 8              # halo width (steps between refreshes)
W = SC + K         # tile width per partition
FLAT = 128 * T * W  # host-pre-tiled input: [partition, t, w] contiguous

R = 200            # max DP rows per DMA/ln chunk
NEG = -1.0e30

_prog_cache = {}
_fused_op = None


# --------------------------------------------------------------------------
# custom DVE op: out[k] = in1[k] + max(in0[k], in0[k-1])
# --------------------------------------------------------------------------
def _build_uops():
    from concourse.dve_uop import (
        DISABLE,
        ENABLE,
        AluInp,
        AluOp,
        InpSel,
        OutPath,
        OutSel,
        Trigger,
        UopConfig,
        UopDpConfig,
    )

    def dp_default():
        return [UopDpConfig() for _ in range(8)]

    seed = UopConfig()
    seed.enable_input(InpSel.SRC_0, 1)
    seed.enable_input(InpSel.SRC_1, 2)
    seed.enable_input(InpSel.MAX_NEG, 3)
    seed.trigger = (Trigger.COUNT, Trigger.NONE, Trigger.NONE)
    seed.repeat_count = 1
    seed.next_uop = (1, 0, 0)
    seed.require_inp0 = DISABLE
    seed.require_inp1 = DISABLE
    seed.datapath_config = dp_default()
    b0 = seed.datapath_config[0]
    b0.enable_alu(AluOp.BYPASS, AluInp.PREV_DELAY_2, AluInp.PREV_DELAY_2)
    b0.swap_enable = ENABLE
    b0.pass_through_delay(0, 1, 2)
    for k in range(1, 8):
        seed.datapath_config[k].pass_through_alu()
        seed.datapath_config[k].pass_through_delay(0, 1, 2)

    st = UopConfig()
    st.enable_input(InpSel.SRC_0, 1)   # lp -> PREV_DELAY_0 at blk0
    st.enable_input(InpSel.SRC_1, 2)   # la -> PREV_DELAY_1 at blk0
    st.enable_input(InpSel.MAX_NEG, 3)
    st.trigger = (Trigger.SRC_TENSOR_DONE, Trigger.NONE, Trigger.NONE)
    st.next_uop = (0, 0, 0)
    st.require_inp0 = ENABLE
    st.require_inp1 = ENABLE
    st.enable_output(OutSel.ALU_OUT, OutPath.WR0_LO)
    st.datapath_config = dp_default()
    d = st.datapath_config
    d[0].enable_alu(AluOp.BYPASS, AluInp.CURR_SWAP_OUT, AluInp.PREV_DELAY_0)
    d[0].swap_enable = ENABLE
    d[0].pass_through_delay(0, 1)
    d[1].enable_alu(AluOp.MAX, AluInp.PREV_ALU_OUT, AluInp.PREV_DELAY_0)
    d[1].pass_through_delay(1)
    d[2].enable_alu(AluOp.ADD, AluInp.PREV_ALU_OUT, AluInp.PREV_DELAY_1)
    for k in range(3, 8):
        d[k].pass_through_alu()
    return [seed, st]


def _build_pair_uops():
    """2-row op: out2[k] = la2[k] + max(out1[k], out1[k-1]),
    out1[k] = la1[k] + max(lp[k], lp[k-1]).  Elements alternate uop A
    (computes out1) / uop B (computes out2, writes); per-stage config
    travels with each element.  in0 = lp duplicated [P,W,2]; in1 =
    (la1[k], la2[k]) interleaved [P,W,2]."""
    from concourse.dve_uop import (
        DISABLE,
        ENABLE,
        AluInp,
        AluOp,
        InpSel,
        OutPath,
        OutSel,
        Trigger,
        UopConfig,
        UopDpConfig,
    )

    def dp_default():
        return [UopDpConfig() for _ in range(8)]

    seed = UopConfig()
    seed.enable_input(InpSel.SRC_0, 1)
    seed.enable_input(InpSel.SRC_1, 2)
    seed.enable_input(InpSel.MAX_NEG, 3)
    seed.trigger = (Trigger.COUNT, Trigger.NONE, Trigger.NONE)
    seed.repeat_count = 4
    seed.next_uop = (1, 0, 0)
    seed.require_inp0 = DISABLE
    seed.require_inp1 = DISABLE
    seed.datapath_config = dp_default()
    sd = seed.datapath_config
    sd[0].enable_alu(AluOp.BYPASS, AluInp.PREV_DELAY_2, AluInp.PREV_DELAY_2)
    sd[0].swap_enable = ENABLE
    sd[0].pass_through_delay(0, 1, 2)
    for k in range(1, 8):
        sd[k].pass_through_alu()
        sd[k].pass_through_delay(0, 1, 2)
    sd[3].enable_alu(AluOp.BYPASS, AluInp.PREV_DELAY_2, AluInp.PREV_DELAY_2)
    sd[3].swap_enable = ENABLE

    def phase_uop(is_a):
        u = UopConfig()
        u.enable_input(InpSel.SRC_0, 1)
        u.enable_input(InpSel.SRC_1, 2)
        u.enable_input(InpSel.MAX_NEG, 3)
        u.trigger = (Trigger.SRC_TENSOR_DONE, Trigger.COUNT, Trigger.NONE)
        u.repeat_count = 1
        u.next_uop = (0, 2 if is_a else 1, 0)
        u.require_inp0 = ENABLE
        u.require_inp1 = ENABLE
        d = u.datapath_config = dp_default()
        d[0].enable_alu(AluOp.BYPASS, AluInp.CURR_SWAP_OUT, AluInp.PREV_DELAY_0)
        d[0].swap_enable = ENABLE if is_a else DISABLE
        d[0].pass_through_delay(0, 1)
        if is_a:
            d[1].enable_alu(AluOp.MAX, AluInp.PREV_ALU_OUT, AluInp.PREV_DELAY_0)
            d[1].pass_through_delay(1)
            d[2].enable_alu(AluOp.ADD, AluInp.PREV_ALU_OUT, AluInp.PREV_DELAY_1)
            d[3].enable_alu(AluOp.BYPASS, AluInp.CURR_SWAP_OUT, AluInp.PREV_ALU_OUT)
            d[3].swap_enable = ENABLE
            d[4].pass_through_alu()
            d[5].pass_through_alu()
        else:
            d[1].pass_through_alu()
            d[1].pass_through_delay(1)
            d[2].pass_through_alu()
            d[2].pass_through_delay(1)
            d[3].enable_alu(AluOp.BYPASS, AluInp.CURR_SWAP_OUT, AluInp.CURR_SWAP_OUT)
            d[3].pass_through_delay(1)
            d[4].enable_alu(AluOp.MAX, AluInp.PREV_ALU_OUT, AluInp.CURR_ALU_OUT)
            d[4].pass_through_delay(1)
            d[5].enable_alu(AluOp.ADD, AluInp.PREV_ALU_OUT, AluInp.PREV_DELAY_1)
        d[6].pass_through_alu()
        d[7].pass_through_alu()
        if not is_a:
            u.enable_output(OutSel.ALU_OUT, OutPath.WR0_LO)
        return u

    return [seed, phase_uop(True), phase_uop(False)]


def _build_quad_uops():
    """4-row op: four chained row updates per instruction.  in0 = lp x4
    dup [P,W,4]; in1 = (la1..la4) interleaved [P,W,4].  Element phases
    A/B/C/D; lag-1 values via CURR_ALU_OUT (same stage, previous raw slot)
    and BYPASS relay chains across phases.  Row r is computed by phase r
    at stages (2r, 2r+1); only phase D writes."""
    from concourse.dve_uop import (
        DISABLE,
        ENABLE,
        AluInp,
        AluOp,
        InpSel,
        OutPath,
        OutSel,
        Trigger,
        UopConfig,
        UopDpConfig,
    )

    PREV = AluInp.PREV_ALU_OUT
    CURR = AluInp.CURR_ALU_OUT
    L0 = AluInp.PREV_DELAY_0
    L1 = AluInp.PREV_DELAY_1

    def dp_default():
        return [UopDpConfig() for _ in range(8)]

    seed = UopConfig()
    seed.enable_input(InpSel.SRC_0, 1)
    seed.enable_input(InpSel.SRC_1, 2)
    seed.trigger = (Trigger.COUNT, Trigger.NONE, Trigger.NONE)
    seed.repeat_count = 1
    seed.next_uop = (1, 0, 0)
    seed.require_inp0 = DISABLE
    seed.require_inp1 = DISABLE
    seed.datapath_config = dp_default()
    for k in range(8):
        seed.datapath_config[k].pass_through_alu()

    def phase_uop(phase):
        u = UopConfig()
        u.enable_input(InpSel.SRC_0, 1)
        u.enable_input(InpSel.SRC_1, 2)
        u.trigger = (Trigger.SRC_TENSOR_DONE, Trigger.COUNT, Trigger.NONE)
        u.repeat_count = 1
        u.next_uop = (0, 1 + ((phase + 1) % 4), 0)
        u.require_inp0 = ENABLE
        u.require_inp1 = ENABLE
        d = u.datapath_config = dp_default()
        A, B, C, D = (phase == 0), (phase == 1), (phase == 2), (phase == 3)
        if A:
            d[0].enable_alu(AluOp.MAX, L0, CURR)
        else:
            d[0].enable_alu(AluOp.BYPASS, L0, L0)
        d[0].pass_through_delay(1)
        if A:
            d[1].enable_alu(AluOp.ADD, PREV, L1)
        else:
            d[1].enable_alu(AluOp.BYPASS, CURR, CURR)
        d[1].pass_through_delay(1)
        if A:
            d[2].enable_alu(AluOp.BYPASS, CURR, CURR)
        elif B:
            d[2].enable_alu(AluOp.MAX, PREV, CURR)
        else:
            d[2].enable_alu(AluOp.BYPASS, PREV, PREV)
        d[2].pass_through_delay(1)
        if B:
            d[3].enable_alu(AluOp.ADD, PREV, L1)
        else:
            d[3].enable_alu(AluOp.BYPASS, CURR, CURR)
        d[3].pass_through_delay(1)
        if C:
            d[4].enable_alu(AluOp.MAX, PREV, CURR)
        elif D:
            d[4].enable_alu(AluOp.BYPASS, PREV, PREV)
        else:
            d[4].enable_alu(AluOp.BYPASS, CURR, CURR)
        d[4].pass_through_delay(1)
        if C:
            d[5].enable_alu(AluOp.ADD, PREV, L1)
        else:
            d[5].enable_alu(AluOp.BYPASS, CURR, CURR)
        d[5].pass_through_delay(1)
        if D:
            d[6].enable_alu(AluOp.MAX, PREV, CURR)
        elif A:
            d[6].enable_alu(AluOp.BYPASS, PREV, PREV)
        else:
            d[6].enable_alu(AluOp.BYPASS, CURR, CURR)
        d[6].pass_through_delay(1)
        if D:
            d[7].enable_alu(AluOp.ADD, PREV, L1)
            u.enable_output(OutSel.ALU_OUT, OutPath.WR0_LO)
        else:
            d[7].enable_alu(AluOp.BYPASS, PREV, PREV)
        return u

    return [seed] + [phase_uop(p) for p in range(4)]


class _CustomOp:
    subdim = False

    def __init__(self, name, build):
        from concourse.dve_spec import Spec, Src0, Src1

        self.name = name
        self._build = build
        self.spec = Spec(body=Src0 + Src1, reference=None)
        self._cache = {}

    def compile(self, ver):
        from concourse.dve_uop import DveOpSpec

        if ver not in self._cache:
            from concourse.dve_ops import get_dve_sub_opcode

            self._cache[ver] = DveOpSpec(
                name=self.name,
                opcode=get_dve_sub_opcode(self.name),
                uops=self._build(),
                rd1_en=True,
            )
        return self._cache[ver]


def _register_op(name, build):
    import concourse.dve_ops as dve_ops

    for o in dve_ops.OPS:
        if o.name == name:
            return o
    op = _CustomOp(name, build)
    dve_ops.OPS.append(op)
    dve_ops._SUB_OPCODE_FOR_NAME[name] = (
        max(dve_ops._SUB_OPCODE_FOR_NAME.values()) + 1
    )
    assert dve_ops._SUB_OPCODE_FOR_NAME[name] < 0x20
    return op


def _get_fused_op():
    return _register_op("MAS_STEP_ANT", _build_uops)


def _get_pair_op():
    return _register_op("MAS_PAIR_ANT", _build_pair_uops)


def _get_quad_op():
    return _register_op("MAS_QUAD_ANT", _build_quad_uops)


# --------------------------------------------------------------------------
# program
# --------------------------------------------------------------------------
def _chunk_plan(tmax):
    """Progressive chunk sizes so the DP starts early.  First chunk is 17
    rows (t=0..16 -> 16 DP steps); later chunks even-sized, so DP step
    parity stays aligned with row pairs and K=8 refresh boundaries."""
    plan = []
    r0 = 0
    for nr in (17, 16, 32, 64, 128):
        if r0 >= tmax:
            return plan
        nr = min(nr, tmax - r0)
        plan.append((r0, nr))
        r0 += nr
    while r0 < tmax:
        nr = min(R, tmax - r0)
        plan.append((r0, nr))
        r0 += nr
    return plan


def _build_program(tmax):
    import concourse.bacc as bacc
    import concourse.bass as bass
    import concourse.mybir as mybir
    from concourse.tile import TileContext

    op1 = _get_fused_op()
    op2 = _get_pair_op()
    op4 = _get_quad_op()
    f32 = mybir.dt.float32
    nc = bacc.Bacc("TRN2", target_bir_lowering=False, debug=False)
    attn_d = nc.dram_tensor("attn", [FLAT], f32, kind="ExternalInput")
    out_d = nc.dram_tensor("res", [128, 1], f32, kind="ExternalOutput")

    shuffle_mask = [31] + list(range(31))  # dest p <- src p-1 within quadrant

    with TileContext(nc) as tc:
        with (
            tc.tile_pool(name="la", bufs=3) as lap,
            tc.tile_pool(name="state", bufs=1) as sp,
        ):
            lp = sp.tile([128, W], f32, tag="lp")
            res = sp.tile([128, 1], f32, tag="res")
            in0_pair = lp[:, 0:W].unsqueeze(2).broadcast_to([128, W, 2])
            in0_quad = lp[:, 0:W].unsqueeze(2).broadcast_to([128, W, 4])

            for ci, (r0, nr) in enumerate(_chunk_plan(tmax)):
                la = lap.tile([128, R * W], f32, tag="la")
                # host pre-tiled layout: one fully-contiguous run per partition
                nc.sync.dma_start(
                    out=la[:, 0 : nr * W],
                    in_=bass.AP(attn_d, r0 * W, [[T * W, 128], [1, nr * W]]),
                )
                nc.scalar.activation(
                    la[:, 0 : nr * W], la[:, 0 : nr * W],
                    mybir.ActivationFunctionType.Ln,
                )
                if ci == 0:
                    nc.vector.memset(lp[:, :], NEG)
                    for b in range(BPC):
                        p = GROUP * b
                        nc.vector.tensor_copy(
                            lp[p : p + 1, K : K + 1], la[p : p + 1, K : K + 1]
                        )
                start_r = 1 if ci == 0 else 0
                r = start_r
                while r < nr:
                    i = r0 + r - 1  # step index, 0-based
                    if i > 0 and i % K == 0:
                        nc.vector.stream_shuffle(
                            lp[:, 0:K], lp[:, W - K : W], mask=shuffle_mask
                        )
                    if r + 3 < nr:
                        nc.vector._custom_dve(
                            op4,
                            out=lp[:, 0:W],
                            in0=in0_quad,
                            in1=la[:, r * W : (r + 4) * W].rearrange(
                                "p (four w) -> p w four", four=4
                            ),
                        )
                        r += 4
                    elif r + 1 < nr:
                        nc.vector._custom_dve(
                            op2,
                            out=lp[:, 0:W],
                            in0=in0_pair,
                            in1=la[:, r * W : (r + 2) * W].rearrange(
                                "p (two w) -> p w two", two=2
                            ),
                        )
                        r += 2
                    else:
                        nc.vector._custom_dve(
                            op1,
                            out=lp[:, 0:W],
                            in0=lp[:, 0:W],
                            in1=la[:, r * W : (r + 1) * W],
                        )
                        r += 1

            nc.vector.reduce_max(
                res[:, 0:1], lp[:, K:W], axis=mybir.AxisListType.X
            )
            nc.sync.dma_start(out=out_d.ap(), in_=res[:, 0:1])

    nc.compile()
    return nc


def _prep_shards(attn, in_lens, out_lens):
    """Per-core masked + pre-tiled flat input buffers.

    Device layout [128, T, W]: partition 32b+s holds attn[b, t, s*16-K+w]
    (0.0 outside [0, 400) -> ln = -inf).  Partitions 25..31 of each quadrant
    stay 0.0, keeping quadrants isolated through the halo-rotate refresh."""
    in_maps = []
    pad = K + S + W  # padded column axis: [-K, S + W)
    for core in range(N_CORES):
        sh = np.zeros((BPC, T, pad), np.float32)
        sh[:, :, K : K + S] = attn[core * BPC : (core + 1) * BPC, 0]
        for b in range(BPC):
            ob = int(out_lens[core * BPC + b])
            ib = int(in_lens[core * BPC + b])
            keep = sh[b, ob - 1, K + ib - 1]
            sh[b, ob - 1, K : K + S] = 0.0   # la -> -inf
            sh[b, ob - 1, K + ib - 1] = keep
            sh[b, ob:, K : K + S] = 1.0      # la -> 0
        flat = np.zeros((128, T, W), np.float32)
        for b in range(BPC):
            win = np.lib.stride_tricks.sliding_window_view(sh[b], W, axis=1)
            flat[GROUP * b : GROUP * b + PS] = win[:, ::SC, :][:, :PS].transpose(
                1, 0, 2
            )
        in_maps.append({"attn": flat.ravel()})
    return in_maps


def _run(attn, in_lens, out_lens, trace=False):
    from concourse import bass_utils

    tmax = int(np.max(out_lens))
    if tmax not in _prog_cache:
        _prog_cache[tmax] = _build_program(tmax)
    nc = _prog_cache[tmax]
    in_maps = _prep_shards(attn, in_lens, out_lens)
    return bass_utils.run_bass_kernel_spmd(
        nc, in_maps, core_ids=list(range(N_CORES)), trace=trace
    )


def kernel(soft_attention, in_lens, out_lens, _trace=False):
    attn = np.asarray(soft_attention, dtype=np.float32)
    inl = np.asarray(in_lens)
    outl = np.asarray(out_lens)
    assert attn.shape == (B, 1, T, S), attn.shape

    res = _run(attn, inl, outl, trace=_trace)

    total = 0.0
    for core in range(N_CORES):
        v = res.results[core]["res"][:, 0]
        for b in range(BPC):
            total += float(np.max(v[GROUP * b : GROUP * b + PS]))
    count = float(np.sum(outl))
    out = np.array(-total / count, dtype=np.float32)
    if _trace:
        return out, res
    return out
